# revision 1
# baseline (speedup 1.0000x reference)
"""Trainium2 Bass kernel for nn_EpipolarWarpOperator (B=8, C=320, H=W=64, S=3).

Sharding: pure data parallelism — one batch element per NeuronCore (8 cores).

Per-core pipeline (fp16 on-chip storage, fp32 PSUM accumulate):
  host: epipolar geometry -> bilinear corner indices/weights; samples sorted
        by y-group; S matrix [128, 20480] (4 nnz/col, bilinear*1/3, padded to
        128-aligned groups); slab row-gather indices; unsort gather indices.
  A: slab(g_b) = x^T rows [64g, 64g+128) fetched by indexed dma_gather from
     HBM; val[128 tok, 320 ch] = S_b.T @ slab  (PE matmul per 128-token block)
  B: unsort: SBUF-source transpose dma_gather of val rows by sorted position
     (per sample s) -> channel-major [128, 3, npix]; mean over s on DVE,
     written into a zero-padded 66x66 layout.
  C: 3x3 conv as 9 shifted matmuls over (mchunk, kchunk, tap), bias+ReLU on
     ACT, DMA out.
"""

import numpy as np

B, C, H, W = 8, 320, 64, 64
NUM_SAMPLES = 3
HW = H * W
NBLK = 160            # fixed token-block budget (>= 12288 + 64*127 padded)
NPAD = NBLK * 128
CPAD = 384            # channel pad so gather elem = 768B (mult of 256)
XROWS = 4224          # x^T rows incl. pad (max slab row 4159)
PW, PH = W + 2, H + 2
MB = [(0, 128), (128, 128), (256, 64)]   # channel chunking

A_CH = 8              # blocks per phase-A chunk
B_CH = 512            # pixels per phase-B chunk
assert NBLK % A_CH == 0 and HW % B_CH == 0

# ---------------------------------------------------------------- host prep

def _rodrigues_np(rv):
    theta = np.sqrt((rv * rv).sum())
    r = rv / max(theta, 1e-12)
    I = np.eye(3, dtype=np.float32)
    K = np.array([[0, -r[2], r[1]], [r[2], 0, -r[0]], [-r[1], r[0], 0]],
                 dtype=np.float32)
    R = np.cos(theta) * I + (1 - np.cos(theta)) * np.outer(r, r) + np.sin(theta) * K
    return I if theta < 1e-6 else R


def fundamental_np(Ks, Kt, ps, pt):
    Fs = []
    for b in range(Ks.shape[0]):
        Rs = _rodrigues_np(ps[b, :3].astype(np.float32))
        Rt = _rodrigues_np(pt[b, :3].astype(np.float32))
        ts_, tt_ = ps[b, 3:].astype(np.float32), pt[b, 3:].astype(np.float32)
        R_rel = Rs @ Rt.T
        t_rel = ts_ - R_rel @ tt_
        z = np.float32(0)
        skew = np.array([[z, -t_rel[2], t_rel[1]],
                         [t_rel[2], z, -t_rel[0]],
                         [-t_rel[1], t_rel[0], z]], dtype=np.float32)
        E = skew @ R_rel
        inv_Ks = np.linalg.inv(Ks[b].astype(np.float32))
        inv_Kt = np.linalg.inv(Kt[b].astype(np.float32))
        Fs.append(inv_Kt.T @ E @ inv_Ks)
    return np.stack(Fs).astype(np.float32)


def geometry(F):
    k = np.arange(HW)
    px = (k % W).astype(np.float32)
    py = (k // W).astype(np.float32)
    P = np.stack([px, py, np.ones_like(px)])
    lines = F.T.astype(np.float32) @ P
    a, b_, c = lines[0], lines[1], lines[2]
    W1, H1 = np.float32(W - 1), np.float32(H - 1)
    EPS = np.float32(1e-10)
    x1 = np.clip(-c / (a + EPS), 0.0, W1)
    x2 = np.clip(-(b_ * H1 + c) / (a + EPS), 0.0, W1)
    y1 = np.clip(-c / (b_ + EPS), 0.0, H1)
    y2 = np.clip(-(a * W1 + c) / (b_ + EPS), 0.0, H1)
    t = np.linspace(0.0, 1.0, NUM_SAMPLES, dtype=np.float32)
    sx = x1[:, None] * (1 - t) + x2[:, None] * t
    sy = y1[:, None] * (1 - t) + y2[:, None] * t
    x0 = np.floor(sx)
    y0 = np.floor(sy)
    wx = (sx - x0).astype(np.float32)
    wy = (sy - y0).astype(np.float32)
    x0i = np.clip(x0, 0, W - 1).astype(np.int32)
    y0i = np.clip(y0, 0, H - 1).astype(np.int32)
    return x0i, y0i, wx, wy


def build_sort(x0i, y0i, wx, wy):
    flat_y = y0i.reshape(-1)
    order = np.argsort(flat_y, kind='stable')
    S = np.zeros((128, NPAD), dtype=np.float32)
    pos = np.zeros(HW * NUM_SAMPLES, dtype=np.int32)
    blk_g = np.zeros(NBLK, dtype=np.int32)
    cur = 0
    x0f = x0i.reshape(-1)
    wxf = wx.reshape(-1)
    wyf = wy.reshape(-1)
    third = np.float32(1.0 / 3.0)
    for g in range(H):
        sel = order[flat_y[order] == g]
        n = sel.size
        if n == 0:
            continue
        cols = cur + np.arange(n)
        pos[sel] = cols
        x0s = x0f[sel]
        wxs = wxf[sel]
        wys = wyf[sel]
        x1s = np.minimum(x0s + 1, W - 1)
        np.add.at(S, (x0s, cols), (1 - wys) * (1 - wxs) * third)
        np.add.at(S, (x1s, cols), (1 - wys) * wxs * third)
        np.add.at(S, (64 + x0s, cols), wys * (1 - wxs) * third)
        np.add.at(S, (64 + x1s, cols), wys * wxs * third)
        nb_lo = cur // 128
        cur = ((cur + n + 127) // 128) * 128
        blk_g[nb_lo:cur // 128] = g
    assert cur <= NPAD, cur
    return S.astype(np.float16), blk_g, pos.reshape(HW, NUM_SAMPLES)


def wrap16(idx, n):
    t = idx.astype(np.int16).reshape(n // 16, 16).T
    return np.tile(t, (8, 1)).copy()


def prep_batch(xb, F):
    x0i, y0i, wx, wy = geometry(F)
    S, blk_g, pos = build_sort(x0i, y0i, wx, wy)
    xt = np.zeros((XROWS, CPAD), dtype=np.float16)
    xt[:HW, :C] = xb.reshape(C, HW).T.astype(np.float16)
    slab_idx = (64 * blk_g[:, None] + np.arange(128)[None, :]).reshape(-1)
    slab_idx16 = wrap16(slab_idx, NBLK * 128)
    gat = np.concatenate(
        [wrap16(pos[:, s], HW) for s in range(NUM_SAMPLES)], axis=1)
    return dict(xt=xt, S=S, slab_idx=slab_idx16, gat_idx=gat)


def prep_weights(conv_w, conv_b):
    Wl = np.zeros((128, 3 * 9 * C), dtype=np.float16)
    for kc, (koff, ksz) in enumerate(MB):
        for tap in range(9):
            dy, dx = tap // 3 - 1, tap % 3 - 1
            for moff, msz in MB:
                blk = conv_w[moff:moff + msz, koff:koff + ksz, dy + 1, dx + 1]
                Wl[0:ksz, kc * 9 * C + tap * C + moff: kc * 9 * C + tap * C
                   + moff + msz] = blk.T.astype(np.float16)
    bias = np.zeros((128, 3), dtype=np.float32)
    for mc, (moff, msz) in enumerate(MB):
        bias[0:msz, mc] = conv_b[moff:moff + msz].astype(np.float32)
    return Wl, bias


# ------------------------------------------------------------- bass program

_NC_CACHE = {}


def build_program(reps=1):
    if reps in _NC_CACHE:
        return _NC_CACHE[reps]
    import concourse.bacc as bacc
    import concourse.mybir as mybir
    from concourse.tile import TileContext

    fp16 = mybir.dt.float16
    f32 = mybir.dt.float32
    i16 = mybir.dt.int16

    nc = bacc.Bacc(target_bir_lowering=False)
    xt = nc.dram_tensor("xt", [XROWS, CPAD], fp16, kind="ExternalInput")
    S = nc.dram_tensor("s_mat", [128, NPAD], fp16, kind="ExternalInput")
    sidx_d = nc.dram_tensor("slab_idx", [128, NBLK * 128 // 16], i16,
                            kind="ExternalInput")
    gidx_d = nc.dram_tensor("gat_idx", [128, 3 * HW // 16], i16,
                            kind="ExternalInput")
    wl_d = nc.dram_tensor("wl", [128, 3 * 9 * C], fp16, kind="ExternalInput")
    bias_d = nc.dram_tensor("bias", [128, 3], f32, kind="ExternalInput")
    out_d = nc.dram_tensor("out", [C, HW], f32, kind="ExternalOutput")

    with TileContext(nc) as tc:
        with tc.tile_pool(name="const", bufs=1) as constp:
            wl = constp.tile([128, 3 * 9 * C], fp16)
            nc.sync.dma_start(out=wl[:], in_=wl_d[:])
            bias_t = constp.tile([128, 3], f32)
            nc.sync.dma_start(out=bias_t[:], in_=bias_d[:])
            sidx = constp.tile([128, NBLK * 128 // 16], i16)
            nc.sync.dma_start(out=sidx[:], in_=sidx_d[:])
            gidx = constp.tile([128, 3 * HW // 16], i16)
            nc.sync.dma_start(out=gidx[:], in_=gidx_d[:])

            def body(_it):
                with tc.tile_pool(name="val", bufs=1) as valp:
                    val = valp.tile([128, NBLK * CPAD], fp16)
                    # zero the channel-pad region of every rank stripe
                    val3 = val.rearrange("p (b c) -> p b c", c=CPAD)
                    nc.gpsimd.memset(val3[:, :, C:CPAD], 0.0)

                    # ---- phase A: sampling matmuls ----
                    with tc.tile_pool(name="slab", bufs=3) as slabp, \
                         tc.tile_pool(name="smat", bufs=3) as smatp, \
                         tc.tile_pool(name="psA", bufs=6, space="PSUM") as psA:
                        for chk in range(NBLK // A_CH):
                            nidx = A_CH * 128
                            slab = slabp.tile([128, A_CH, CPAD], fp16)
                            nc.gpsimd.dma_gather(
                                out_ap=slab[:],
                                in_ap=xt[:],
                                idxs_ap=sidx[:, chk * nidx // 16:
                                             (chk + 1) * nidx // 16],
                                num_idxs=nidx,
                                num_idxs_reg=nidx,
                                elem_size=CPAD,
                            )
                            smat = smatp.tile([128, A_CH * 128], fp16)
                            nc.sync.dma_start(
                                out=smat[:],
                                in_=S[:, chk * nidx:(chk + 1) * nidx])
                            for b in range(A_CH):
                                blk = chk * A_CH + b
                                ps = psA.tile([128, C], f32)
                                nc.tensor.matmul(
                                    ps[:],
                                    smat[:, b * 128:(b + 1) * 128],
                                    slab[:, b, 0:C],
                                    start=True, stop=True)
                                eng = nc.vector if b % 2 == 0 else nc.scalar
                                if b % 2 == 0:
                                    nc.vector.tensor_copy(
                                        val3[:, blk, 0:C], ps[:])
                                else:
                                    nc.scalar.copy(val3[:, blk, 0:C], ps[:])

                    # ---- phase B: unsort + mean -> padded layout ----
                    with tc.tile_pool(name="samp", bufs=1) as sampp:
                        sampled = sampp.tile([128, 3 * PH * PW], fp16)
                        nc.gpsimd.memset(sampled[:], 0.0)
                        smp4 = sampled.rearrange("p (k r c) -> p k r c",
                                                 k=3, r=PH)
                        with tc.tile_pool(name="gout", bufs=2) as goutp:
                            rows_per = B_CH // W
                            for q in range(HW // B_CH):
                                gs = []
                                for s in range(NUM_SAMPLES):
                                    g = goutp.tile([128, 3, B_CH], fp16,
                                                   tag=f"g{s}")
                                    nc.gpsimd.dma_gather(
                                        out_ap=g[:],
                                        in_ap=val[:],
                                        idxs_ap=gidx[:, (s * HW + q * B_CH) // 16:
                                                     (s * HW + (q + 1) * B_CH) // 16],
                                        num_idxs=B_CH,
                                        num_idxs_reg=B_CH,
                                        elem_size=CPAD,
                                        transpose=True,
                                        sbuf_tokens_per_rank=128,
                                        sbuf_free_dim_per_rank=CPAD * 2,
                                    )
                                    gs.append(g)
                                tmp = goutp.tile([128, 3 * B_CH], fp16,
                                                 tag="tmp")
                                nc.vector.tensor_add(
                                    tmp[:],
                                    gs[0].rearrange("p k n -> p (k n)"),
                                    gs[1].rearrange("p k n -> p (k n)"))
                                r0 = 1 + q * rows_per
                                nc.vector.tensor_add(
                                    smp4[:, :, r0:r0 + rows_per, 1:1 + W],
                                    tmp.rearrange("p (k r c) -> p k r c",
                                                  k=3, c=W),
                                    gs[2].rearrange("p k (r c) -> p k r c",
                                                    c=W))

                        # ---- phase C: 3x3 conv + bias + relu ----
                        with tc.tile_pool(name="psC", bufs=4, space="PSUM") \
                                as psC, \
                             tc.tile_pool(name="outp", bufs=3) as outp:
                            NCOL = 512
                            rows_n = NCOL // W
                            for mc, (moff, msz) in enumerate(MB):
                                for r in range(HW // NCOL):
                                    ps = psC.tile([128, NCOL], f32)
                                    n_mm = 27
                                    i_mm = 0
                                    for tap in range(9):
                                        dy, dx = tap // 3 - 1, tap % 3 - 1
                                        for kc, (koff, ksz) in enumerate(MB):
                                            rhs = smp4[0:ksz, kc,
                                                       1 + dy + r * rows_n:
                                                       1 + dy + r * rows_n + rows_n,
                                                       1 + dx:1 + dx + W]
                                            lhsT = wl[0:ksz,
                                                      kc * 9 * C + tap * C + moff:
                                                      kc * 9 * C + tap * C + moff + msz]
                                            nc.tensor.matmul(
                                                ps[0:msz],
                                                lhsT, rhs,
                                                start=(i_mm == 0),
                                                stop=(i_mm == n_mm - 1))
                                            i_mm += 1
                                    ot = outp.tile([128, NCOL], f32)
                                    import concourse.mybir as _mb
                                    nc.scalar.activation(
                                        ot[0:msz], ps[0:msz],
                                        _mb.ActivationFunctionType.Relu,
                                        bias=bias_t[0:msz, mc:mc + 1])
                                    nc.sync.dma_start(
                                        out=out_d[moff:moff + msz,
                                                  r * NCOL:(r + 1) * NCOL],
                                        in_=ot[0:msz])

            if reps == 1:
                body(0)
            else:
                with tc.For_i(0, reps, 1) as it:
                    body(it)

    nc.finalize()
    _NC_CACHE[reps] = nc
    return nc


# ---------------------------------------------------------------- interface

def make_in_maps(x, source_intrinsics, target_intrinsics, source_pose,
                 target_pose, conv_w, conv_b):
    F = fundamental_np(source_intrinsics, target_intrinsics,
                       source_pose, target_pose)
    Wl, bias = prep_weights(conv_w, conv_b)
    in_maps = []
    for b in range(B):
        d = prep_batch(x[b], F[b])
        in_maps.append({
            "xt": d["xt"], "s_mat": d["S"], "slab_idx": d["slab_idx"],
            "gat_idx": d["gat_idx"], "wl": Wl, "bias": bias,
        })
    return in_maps


def kernel(x, source_intrinsics, target_intrinsics, source_pose,
           target_pose, conv_w, conv_b, _reps=1):
    from concourse.bass_utils import run_bass_kernel_spmd
    x = np.asarray(x, dtype=np.float32)
    in_maps = make_in_maps(
        x, np.asarray(source_intrinsics), np.asarray(target_intrinsics),
        np.asarray(source_pose), np.asarray(target_pose),
        np.asarray(conv_w, dtype=np.float32), np.asarray(conv_b, dtype=np.float32))
    nc = build_program(_reps)
    res = run_bass_kernel_spmd(nc, in_maps, list(range(8)))
    out = np.stack([res.results[i]["out"].reshape(C, H, W) for i in range(8)])
    return out.astype(np.float32)
